# revision 1
# baseline (speedup 1.0000x reference)
# Bass/Trainium2 kernel for DSGR message-passing layer (8-core SPMD).
#
# Strategy: shard user/item node axes 8-way; weights + embedding tables are
# replicated. Each core builds the full bf16 uh/ih projection tables
# (stationary = host-cast bf16 feature block so psum comes out node-major),
# writes them to DRAM, and fetches each 128-node tile's mailboxes with
# batched dma_gather calls: 51 slots per node (50 time-sorted neighbors +
# the argmax-time neighbor), chunked 7 slots (896 descriptors) per call to
# stay under the 1024-descriptor SWDGE ring, round-robined over 4 SWDGE
# queues. The time-sort permutation (the reference's double argsort) is a
# pure function of the host-known time tensor and is folded into the gather
# indices on the host, so the mailbox arrives already time-ordered and all
# rank/permutation compute disappears from the device.
#
# Per-tile compute (DVE-bound, ~24us/tile): dot products and weighted sums
# run as bf16 2x-mode DVE ops -- inner-broadcast operands use a
# pair-duplicated alpha layout [n, r, 2] to keep the packed-inner-axis 2x
# mode; dots reduce via a bf16 add tree (64/32/16) + short f32 reduce; the
# alpha-weighted mailbox sums stream through the PE as 13 accumulated
# identity matmuls into a 4-way-striped psum. The two softmax chains are
# interleaved so the Scalar-engine exp latency hides under DVE work.
#
# Gate/update weights are pre-folded on the host:
#   out = tanh(h_long@A + h_short@B + alpha@E + feat@Ub)
#   A = Gt@Ut, B = Gb@Ut, E = embk@Gt@Ut, plus P = feat @ (W@embT).
import os
import numpy as np

D = 128
L = 50
NS = 51                    # mailbox slots kept per node: 50 sorted + last
IDXC = NS * 128 // 16      # idx columns per tile (wrapped int16 layout)
NU = 8192
NI = 8192
NCORES = 8
SCALE = 1.0 / float(np.sqrt(128.0))

_CACHE = {}


def _build_program(sh, nu, ni):
    """Build the (core-uniform) Bass program. sh = shard nodes per side."""
    STAGE = int(os.environ.get("BASSK_STAGE", "4"))
    import concourse.bacc as bacc
    import concourse.mybir as mybir
    import concourse.tile as tile
    from concourse.tile_rust import add_dep_helper

    f32 = mybir.dt.float32
    i16 = mybir.dt.int16
    bf16 = mybir.dt.bfloat16
    Alu = mybir.AluOpType
    Act = mybir.ActivationFunctionType
    AX = mybir.AxisListType

    tpc = sh // 128
    nblk_u = nu // 128
    nblk_i = ni // 128

    NSWQ = int(os.environ.get("BASSK_NSWQ", "4"))
    nc = bacc.Bacc("TRN2", target_bir_lowering=False, debug=False,
                   num_swdge_queues=NSWQ)

    def inp(name, shape, dtype=f32):
        return nc.declare_dram_parameter(name, list(shape), dtype, isOutput=False)

    # ---- inputs ----
    userTb = inp("userTb", [D, nu], bf16)    # replicated, bf16 features (d-major)
    itemTb = inp("itemTb", [D, ni], bf16)
    Wub = inp("Wub", [D, D], bf16)
    Wib = inp("Wib", [D, D], bf16)
    featuT = inp("featuT", [D, sh])          # per-core f32 feature shard
    featiT = inp("featiT", [D, sh])
    featuTb = inp("featuTb", [D, sh], bf16)  # per-core bf16 feature shard
    featiTb = inp("featiTb", [D, sh], bf16)
    WembTu = inp("WembTu", [D, L])           # W_user @ user_date_emb.T
    WembTi = inp("WembTi", [D, L])
    Au = inp("Au", [D, D])                   # Gt@Ut
    Bu = inp("Bu", [D, D])                   # Gb@Ut
    Eu = inp("Eu", [L, D])                   # embk@Gt@Ut
    Ubu = inp("Ubu", [D, D])                 # update rows 128:256
    Ai = inp("Ai", [D, D])
    Bi = inp("Bi", [D, D])
    Ei = inp("Ei", [L, D])
    Ubi = inp("Ubi", [D, D])
    uidx = inp("uidx", [128, tpc * IDXC], i16)   # per-core wrapped gather idxs
    iidx = inp("iidx", [128, tpc * IDXC], i16)
    ident = inp("ident", [D, D])
    identb = inp("identb", [D, D], bf16)

    uout = nc.declare_dram_parameter("uout", [sh, D], f32, isOutput=True)
    iout = nc.declare_dram_parameter("iout", [sh, D], f32, isOutput=True)

    # internal DRAM tables (node-major, bf16)
    uhd = nc.dram_tensor("uhd", [nu, D], bf16)
    ihd = nc.dram_tensor("ihd", [ni, D], bf16)

    with tile.TileContext(nc) as tc:
        with (
            tc.tile_pool(name="const", bufs=1) as constp,
            tc.tile_pool(name="tstage", bufs=8) as tstagep,
            tc.tile_pool(name="psum", bufs=3, space="PSUM") as psump,
            tc.tile_pool(name="psmm", bufs=5, space="PSUM") as psmm,
        ):
            # ---------- load constants ----------
            def load_const(src, shape, dtype=f32):
                t = constp.tile(list(shape), dtype, tag=src.name)
                nc.sync.dma_start(t[:], src[:])
                return t

            # Load order matters: the item table build gates the first
            # user-side gathers, so its dependencies (itemTb, Wib) go first
            # on the Sync DMA queue.
            table_dmas = {"u": [], "i": []}
            with tc.tile_pool(name="tbl", bufs=1) as tblp:
                itemTb_s = tblp.tile([D, ni], bf16, tag="itemTb")
                for q4 in range(4):
                    qs = ni // 4
                    nc.sync.dma_start(itemTb_s[:, q4 * qs:(q4 + 1) * qs],
                                      itemTb[:, q4 * qs:(q4 + 1) * qs])
                Wib_s = load_const(Wib, [D, D], bf16)
                Wub_s = load_const(Wub, [D, D], bf16)
                featuTb_s = load_const(featuTb, [D, sh], bf16)
                featiTb_s = load_const(featiTb, [D, sh], bf16)
                uidx_s = load_const(uidx, [128, tpc * IDXC], i16)
                userTb_s = tblp.tile([D, nu], bf16, tag="userTb")
                for q4 in range(4):
                    qs = nu // 4
                    nc.sync.dma_start(userTb_s[:, q4 * qs:(q4 + 1) * qs],
                                      userTb[:, q4 * qs:(q4 + 1) * qs])
                iidx_s = load_const(iidx, [128, tpc * IDXC], i16)
                featuT_s = load_const(featuT, [D, sh])
                featiT_s = load_const(featiT, [D, sh])
                WembTu_s = load_const(WembTu, [D, L])
                WembTi_s = load_const(WembTi, [D, L])
                Au_s = load_const(Au, [D, D])
                Bu_s = load_const(Bu, [D, D])
                Eu_s = load_const(Eu, [L, D])
                Ubu_s = load_const(Ubu, [D, D])
                Ai_s = load_const(Ai, [D, D])
                Bi_s = load_const(Bi, [D, D])
                Ei_s = load_const(Ei, [L, D])
                Ubi_s = load_const(Ubi, [D, D])
                ident_s = load_const(ident, [D, D])
                identb_s = load_const(identb, [D, D], bf16)

                def build_table(srcTb_s, Wb_s, dstd, nblk, side, use_dve):
                    # use_dve: alternate the psum->bf16 staging copies between
                    # Scalar and Vector and issue the write DMAs from the DVE
                    # HWDGE queue (only safe while DVE is otherwise idle);
                    # Sync's queue is busy streaming the const loads then.
                    for grp in range(nblk // 4):
                        pt = psump.tile([128, 512], f32, tag="ptab")
                        for j in range(4):
                            b = grp * 4 + j
                            nc.tensor.matmul(
                                pt[:, j * 128:(j + 1) * 128],
                                srcTb_s[:, b * 128:(b + 1) * 128], Wb_s[:],
                                start=True, stop=True,
                            )
                        st = tstagep.tile([128, 512], bf16, tag="tstg")
                        if use_dve and grp % 2 == 1:
                            nc.vector.tensor_copy(st[:], pt[:])
                        else:
                            nc.scalar.copy(st[:], pt[:])
                        weng = nc.scalar if use_dve else nc.sync
                        dmai = weng.dma_start(
                            dstd[grp * 512:(grp + 1) * 512, :].rearrange(
                                "(j p) f -> p j f", p=128
                            ),
                            st[:].rearrange("p (j f) -> p j f", f=128),
                        )
                        table_dmas[side].append(dmai)

                def own_proj(featTb_s, Wb_s, nm):
                    h_nm = constp.tile([128, tpc, D], bf16, tag="hnm_" + nm)
                    for t in range(tpc):
                        p1 = psmm.tile([128, D], f32, tag="mm")
                        nc.tensor.matmul(
                            p1[:], featTb_s[:, t * 128:(t + 1) * 128], Wb_s[:],
                            start=True, stop=True,
                        )
                        nc.scalar.copy(h_nm[:, t, :], p1[:])
                    return h_nm

                # PE order: item table (gates user gathers), then own-shard
                # projections, then the user table.
                build_table(itemTb_s, Wib_s, ihd, nblk_i, "i", True)
                uh_nm = own_proj(featuTb_s, Wub_s, 'u')
                ih_nm = own_proj(featiTb_s, Wib_s, 'i')
                build_table(userTb_s, Wub_s, uhd, nblk_u, "u", False)

            # ---------- main per-tile loop ----------
            with (
                tc.tile_pool(name="mtile", bufs=4) as mpool,
                tc.tile_pool(name="prod", bufs=4) as prodp,
                tc.tile_pool(name="tree", bufs=2) as treep,
                tc.tile_pool(name="small", bufs=3) as smallp,
            ):
                sides = (
                    (0, ihd, uh_nm, featuT_s, WembTu_s, Au_s, Bu_s, Eu_s, Ubu_s,
                     uidx_s, uout),
                    (1, uhd, ih_nm, featiT_s, WembTi_s, Ai_s, Bi_s, Ei_s, Ubi_s,
                     iidx_s, iout),
                )
                gq = 0
                for (sidx, tabled, own_nm, featT_s, WembT_s, A_s, B_s, E_s, Ub_s,
                     idx_s, outh) in sides:
                    opp_dmas = table_dmas["i" if sidx == 0 else "u"]
                    for t in range(tpc):
                        r0 = t * 128
                        # -- batched mailbox gather: 52 slots x 128 nodes, in
                        # chunks of 7 slots (896 descriptors, safely under the
                        # 1024-descriptor SWDGE ring capacity) --
                        M = mpool.tile([128, NS, D], bf16, tag="M")
                        for c0 in range(0, NS, 7):
                            cw = min(7, NS - c0)
                            g = nc.gpsimd.dma_gather(
                                M[:, c0:c0 + cw, :], tabled[:, :],
                                idx_s[:, t * IDXC + c0 * 8:
                                      t * IDXC + (c0 + cw) * 8],
                                cw * 128, cw * 128, D,
                                queue_num=gq % NSWQ,
                            )
                            gq += 1
                            for dmai in opp_dmas:
                                add_dep_helper(g.ins, dmai.ins, reason="table RAW")

                        if STAGE == 1:
                            o1 = smallp.tile([128, D], f32, tag="out")
                            nc.vector.tensor_copy(o1[:], M[:, 0, :])
                            nc.sync.dma_start(outh[r0:r0 + 128, :], o1[:])
                            continue

                        # -- P = feat @ (W@embT) -> [128n, L] (psum) --
                        pP = psmm.tile([128, L], f32, tag="mm")
                        nc.tensor.matmul(
                            pP[:], featT_s[:, r0:r0 + 128], WembT_s[:],
                            start=True, stop=True,
                        )

                        # -- m_dot = sum_d M * uh (bf16 tree + short reduce) --
                        prodM = prodp.tile([128, L, D], bf16, tag="prod")
                        nc.vector.tensor_tensor(
                            prodM[:], M[:, 0:L, :],
                            own_nm[:, t, :].unsqueeze(1).broadcast_to([128, L, D]),
                            Alu.mult,
                        )
                        t64 = treep.tile([128, L, 64], bf16, tag="t64")
                        nc.vector.tensor_tensor(
                            t64[:], prodM[:, :, 0:64], prodM[:, :, 64:128], Alu.add
                        )
                        t32 = treep.tile([128, L, 32], bf16, tag="t32")
                        nc.vector.tensor_tensor(
                            t32[:], t64[:, :, 0:32], t64[:, :, 32:64], Alu.add
                        )
                        t16 = treep.tile([128, L, 16], bf16, tag="t16")
                        nc.vector.tensor_tensor(
                            t16[:], t32[:, :, 0:16], t32[:, :, 16:32], Alu.add
                        )
                        m_dot = smallp.tile([128, L], f32, tag="mdot")
                        nc.vector.tensor_reduce(
                            m_dot[:], t16[:], axis=AX.X, op=Alu.add
                        )

                        # -- alpha = softmax(scale*(m_dot + P)) (no max-sub:
                        # |logits| <= ~6 so exp is safe in f32) --
                        e_t = smallp.tile([128, L], f32, tag="e")
                        nc.vector.tensor_tensor(e_t[:], m_dot[:], pP[:], Alu.add)

                        if STAGE == 2:
                            o2 = smallp.tile([128, D], f32, tag="out")
                            nc.vector.memset(o2[:], 0.0)
                            nc.vector.tensor_copy(o2[:, 0:L], e_t[:])
                            nc.sync.dma_start(outh[r0:r0 + 128, :], o2[:])
                            continue

                        ex = smallp.tile([128, L], f32, tag="ex")
                        Z = smallp.tile([128, 1], f32, tag="Z")
                        nc.scalar.activation(
                            ex[:], e_t[:], Act.Exp, scale=SCALE, accum_out=Z[:]
                        )

                        # -- e1 = M . M_last chain interleaved here so the
                        # DVE keeps working while exp1 runs on Scalar --
                        prodE = prodp.tile([128, L, D], bf16, tag="prod")
                        nc.vector.tensor_tensor(
                            prodE[:], M[:, 0:L, :],
                            M[:, L:L + 1, :].broadcast_to([128, L, D]),
                            Alu.mult,
                        )
                        t64b = treep.tile([128, L, 64], bf16, tag="t64")
                        nc.vector.tensor_tensor(
                            t64b[:], prodE[:, :, 0:64], prodE[:, :, 64:128], Alu.add
                        )
                        t32b = treep.tile([128, L, 32], bf16, tag="t32")
                        nc.vector.tensor_tensor(
                            t32b[:], t64b[:, :, 0:32], t64b[:, :, 32:64], Alu.add
                        )
                        t16b = treep.tile([128, L, 16], bf16, tag="t16")
                        nc.vector.tensor_tensor(
                            t16b[:], t32b[:, :, 0:16], t32b[:, :, 16:32], Alu.add
                        )
                        e1 = smallp.tile([128, L], f32, tag="e1")
                        nc.vector.tensor_reduce(
                            e1[:], t16b[:], axis=AX.X, op=Alu.add
                        )
                        ex1 = smallp.tile([128, L], f32, tag="ex1")
                        Z1 = smallp.tile([128, 1], f32, tag="Z1")
                        nc.scalar.activation(
                            ex1[:], e1[:], Act.Exp, scale=SCALE, accum_out=Z1[:]
                        )

                        # -- alpha, h_long --
                        rZ = smallp.tile([128, 1], f32, tag="rZ")
                        nc.vector.reciprocal(rZ[:], Z[:])
                        alphaE = smallp.tile([128, NS, 2], bf16, tag="aE")
                        nc.vector.memset(alphaE[:, L:NS, :], 0.0)
                        nc.vector.tensor_tensor(
                            alphaE[:, 0:L, :],
                            ex[:].unsqueeze(2).broadcast_to([128, L, 2]),
                            rZ[:].broadcast_to([128, L]).unsqueeze(2)
                                .broadcast_to([128, L, 2]),
                            Alu.mult,
                        )
                        alphaf = smallp.tile([128, L], f32, tag="af")
                        nc.vector.tensor_tensor(
                            alphaf[:], ex[:], rZ[:].broadcast_to([128, L]), Alu.mult
                        )
                        prodA = prodp.tile([128, NS, D], bf16, tag="prod")
                        nc.vector.tensor_tensor(
                            prodA[:].rearrange("p r (a b) -> p r a b", b=2),
                            M[:, :, :].rearrange("p r (a b) -> p r a b", b=2),
                            alphaE[:].unsqueeze(2).broadcast_to([128, NS, 64, 2]),
                            Alu.mult,
                        )
                        ph_l = psump.tile([128, 512], f32, tag="ptab")
                        ng4 = (NS + 3) // 4
                        for g4 in range(ng4):
                            gw = min(4, NS - g4 * 4)
                            nc.tensor.matmul(
                                ph_l[:, 0:gw * 128], identb_s[:],
                                prodA[:, g4 * 4:g4 * 4 + gw, :].rearrange(
                                    "p a d -> p (a d)"),
                                start=(g4 == 0), stop=(g4 == ng4 - 1),
                            )

                        # -- alpha1, h_short --
                        rZ1 = smallp.tile([128, 1], f32, tag="rZ1")
                        nc.vector.reciprocal(rZ1[:], Z1[:])
                        alpha1E = smallp.tile([128, NS, 2], bf16, tag="a1E")
                        nc.vector.memset(alpha1E[:, L:NS, :], 0.0)
                        nc.vector.tensor_tensor(
                            alpha1E[:, 0:L, :],
                            ex1[:].unsqueeze(2).broadcast_to([128, L, 2]),
                            rZ1[:].broadcast_to([128, L]).unsqueeze(2)
                                .broadcast_to([128, L, 2]),
                            Alu.mult,
                        )
                        prodS = prodp.tile([128, NS, D], bf16, tag="prod")
                        nc.vector.tensor_tensor(
                            prodS[:].rearrange("p r (a b) -> p r a b", b=2),
                            M[:, :, :].rearrange("p r (a b) -> p r a b", b=2),
                            alpha1E[:].unsqueeze(2).broadcast_to([128, NS, 64, 2]),
                            Alu.mult,
                        )
                        ph_s = psump.tile([128, 512], f32, tag="ptab")
                        for g4 in range(ng4):
                            gw = min(4, NS - g4 * 4)
                            nc.tensor.matmul(
                                ph_s[:, 0:gw * 128], identb_s[:],
                                prodS[:, g4 * 4:g4 * 4 + gw, :].rearrange(
                                    "p a d -> p (a d)"),
                                start=(g4 == 0), stop=(g4 == ng4 - 1),
                            )

                        # -- fold the striped psums down to h_long/h_short --
                        hql = smallp.tile([128, 512], f32, tag="hq")
                        nc.scalar.copy(hql[:], ph_l[:])
                        h2l = smallp.tile([128, 256], f32, tag="h2")
                        nc.vector.tensor_tensor(
                            h2l[:], hql[:, 0:256], hql[:, 256:512], Alu.add
                        )
                        h_long = smallp.tile([128, D], f32, tag="hl")
                        nc.vector.tensor_tensor(
                            h_long[:], h2l[:, 0:128], h2l[:, 128:256], Alu.add
                        )
                        hqs = smallp.tile([128, 512], f32, tag="hqs")
                        nc.scalar.copy(hqs[:], ph_s[:])
                        h2s = smallp.tile([128, 256], f32, tag="h2s")
                        nc.vector.tensor_tensor(
                            h2s[:], hqs[:, 0:256], hqs[:, 256:512], Alu.add
                        )
                        h_short = smallp.tile([128, D], f32, tag="hs")
                        nc.vector.tensor_tensor(
                            h_short[:], h2s[:, 0:128], h2s[:, 128:256], Alu.add
                        )

                        if STAGE == 3:
                            o3 = smallp.tile([128, D], f32, tag="out")
                            nc.vector.tensor_copy(o3[:], h_long[:])
                            nc.sync.dma_start(outh[r0:r0 + 128, :], o3[:])
                            continue

                        # -- transposes for the update matmuls --
                        pT1 = psmm.tile([128, D], f32, tag="mm")
                        nc.tensor.transpose(pT1[:], h_long[:], ident_s[:])
                        hlT = smallp.tile([128, D], f32, tag="hlT")
                        nc.scalar.copy(hlT[:], pT1[:])
                        pT2 = psmm.tile([128, D], f32, tag="mm")
                        nc.tensor.transpose(pT2[:], h_short[:], ident_s[:])
                        hsT = smallp.tile([128, D], f32, tag="hsT")
                        nc.scalar.copy(hsT[:], pT2[:])
                        pT3 = psmm.tile([L, 128], f32, tag="mm")
                        nc.tensor.transpose(pT3[:], alphaf[:], ident_s[:])
                        aT = smallp.tile([L, 128], f32, tag="aT")
                        nc.scalar.copy(aT[:], pT3[:])

                        # -- out = tanh(hl@A + hs@B + alpha@E + feat@Ub) --
                        po = psmm.tile([128, D], f32, tag="mm")
                        nc.tensor.matmul(po[:], hlT[:], A_s[:], start=True, stop=False)
                        nc.tensor.matmul(po[:], hsT[:], B_s[:], start=False, stop=False)
                        nc.tensor.matmul(po[:], aT[:], E_s[:], start=False, stop=False)
                        nc.tensor.matmul(
                            po[:], featT_s[:, r0:r0 + 128], Ub_s[:],
                            start=False, stop=True,
                        )
                        out_s = smallp.tile([128, D], f32, tag="out")
                        nc.scalar.activation(out_s[:], po[:], Act.Tanh)
                        nc.sync.dma_start(outh[r0:r0 + 128, :], out_s[:])

    nc.compile()
    return nc


def _build_ids(nbr, time):
    """[N,L] neighbor ids + times -> [N, NS] gather rows per node:
    slots 0..49 = neighbors sorted most-recent-first (exact double-argsort
    inverse, stable => matches jnp tie semantics), slot 50 = argmax-time
    neighbor (first max, = reference h_short), slot 51 = pad."""
    s = np.argsort(time, axis=1, kind='stable')
    sigma = s[:, ::-1]
    sorted_nbr = np.take_along_axis(nbr, sigma, axis=1)
    last = np.argmax(time, axis=1)
    lastn = np.take_along_axis(nbr, last[:, None], axis=1)
    return np.concatenate([sorted_nbr, lastn], axis=1)


def _wrap_idx(ids):
    """[sh, NS] int -> [128, tpc*IDXC] int16 wrapped for dma_gather."""
    sh = ids.shape[0]
    tpc = sh // 128
    cols = []
    for t in range(tpc):
        blk = ids[t * 128:(t + 1) * 128, :]        # [128, NS]
        req = blk.T.reshape(-1)                    # i = r*128 + p
        w = req.reshape(-1, 16).T                  # [16, IDXC]
        cols.append(np.tile(w, (8, 1)))
    return np.ascontiguousarray(
        np.concatenate(cols, axis=1)).astype(np.int16)


def kernel(**inputs):
    import ml_dtypes
    from concourse.bass_utils import run_bass_kernel_spmd
    bfdt = ml_dtypes.bfloat16

    user = np.asarray(inputs["user"], np.float32)
    item = np.asarray(inputs["item"], np.float32)
    W_user = np.asarray(inputs["W_user"], np.float32)
    W_item = np.asarray(inputs["W_item"], np.float32)
    agu = np.asarray(inputs["agg_gate_user"], np.float32)
    agi = np.asarray(inputs["agg_gate_item"], np.float32)
    upu = np.asarray(inputs["update_user"], np.float32)
    upi = np.asarray(inputs["update_item"], np.float32)
    uemb = np.asarray(inputs["user_date_emb"], np.float32)
    uembk = np.asarray(inputs["user_date_emb_k"], np.float32)
    iemb = np.asarray(inputs["item_date_emb"], np.float32)
    iembk = np.asarray(inputs["item_date_emb_k"], np.float32)
    unbr = np.asarray(inputs["user_nbr"], np.int64)
    untime = np.asarray(inputs["user_nbr_time"], np.int64)
    inbr = np.asarray(inputs["item_nbr"], np.int64)
    intime = np.asarray(inputs["item_nbr_time"], np.int64)

    nu, d = user.shape
    ni = item.shape[0]
    sh = nu // NCORES

    key = (sh, nu, ni)
    if key not in _CACHE:
        _CACHE[key] = _build_program(sh, nu, ni)
    nc = _CACHE[key]

    # host precompute: gather ids (time-sorted + last + pad)
    uids = _build_ids(unbr, untime)      # [NU, NS] (rows into item table)
    iids = _build_ids(inbr, intime)      # [NI, NS] (rows into user table)

    # host-folded weights
    Au = agu[:D] @ upu[:D]
    Bu = agu[D:] @ upu[:D]
    Eu = uembk @ Au
    Ai = agi[:D] @ upi[:D]
    Bi = agi[D:] @ upi[:D]
    Ei = iembk @ Ai
    ident = np.eye(D, dtype=np.float32)

    userT = np.ascontiguousarray(user.T)
    itemT = np.ascontiguousarray(item.T)

    common = dict(
        userTb=userT.astype(bfdt),
        itemTb=itemT.astype(bfdt),
        Wub=W_user.astype(bfdt), Wib=W_item.astype(bfdt),
        WembTu=np.ascontiguousarray(W_user @ uemb.T),
        WembTi=np.ascontiguousarray(W_item @ iemb.T),
        Au=Au, Bu=Bu, Eu=np.ascontiguousarray(Eu),
        Ubu=np.ascontiguousarray(upu[D:]),
        Ai=Ai, Bi=Bi, Ei=np.ascontiguousarray(Ei),
        Ubi=np.ascontiguousarray(upi[D:]),
        ident=ident, identb=ident.astype(bfdt),
    )

    in_maps = []
    for c in range(NCORES):
        su = slice(c * sh, (c + 1) * sh)
        m = dict(common)
        m["featuT"] = np.ascontiguousarray(userT[:, su])
        m["featiT"] = np.ascontiguousarray(itemT[:, su])
        m["featuTb"] = m["featuT"].astype(bfdt)
        m["featiTb"] = m["featiT"].astype(bfdt)
        m["uidx"] = _wrap_idx(uids[su])
        m["iidx"] = _wrap_idx(iids[su])
        in_maps.append(m)

    _LAST_RUN["nc"] = nc
    _LAST_RUN["in_maps"] = in_maps
    res = run_bass_kernel_spmd(nc, in_maps, list(range(NCORES)))
    user_out = np.concatenate([res.results[c]["uout"] for c in range(NCORES)], 0)
    item_out = np.concatenate([res.results[c]["iout"] for c in range(NCORES)], 0)
    return user_out, item_out


_LAST_RUN = {}


def _install_ntff_hook():
    import sys as _sys
    import types as _types
    try:
        from antenv.axon_hooks import get_axon_ntff_profile_hook  # noqa: F401
        return
    except ImportError:
        pass
    if "/root/.axon_site" not in _sys.path:
        _sys.path.insert(0, "/root/.axon_site")
    from trn_agent_boot.trn_boot import _ntff_profile_via_ctypes
    hook = _ntff_profile_via_ctypes("/opt/axon/libaxon_pjrt.so")
    mod = _types.ModuleType("antenv.axon_hooks")
    mod.get_axon_ntff_profile_hook = lambda: hook
    mod.set_axon_ntff_profile_hook = lambda h: None
    _sys.modules["antenv.axon_hooks"] = mod
    import antenv
    antenv.axon_hooks = mod


def bench(n=1):
    """Re-run the cached program with trace=True and return HW exec ns."""
    import time as _t
    from concourse.bass_utils import run_bass_kernel_spmd
    nc = _LAST_RUN["nc"]
    in_maps = _LAST_RUN["in_maps"]
    best = None
    try:
        _install_ntff_hook()
        for _ in range(n):
            res = run_bass_kernel_spmd(
                nc, in_maps, list(range(NCORES)), trace=True
            )
            t = res.exec_time_ns or res.mean_exec_time_ns
            if t and (best is None or t < best):
                best = t
    except Exception as e:
        print("bench trace path failed:", repr(e))
    if best is None:
        # fallback: wall-clock around the execute (includes dispatch)
        for _ in range(3):
            t0 = _t.perf_counter()
            run_bass_kernel_spmd(nc, in_maps, list(range(NCORES)))
            dt = (_t.perf_counter() - t0) * 1e9
            best = dt if best is None or dt < best else best
    return int(best)

